# revision 33
# baseline (speedup 1.0000x reference)
"""Trainium2 Bass kernel for nn_BertCosAttention (B=8, S=2048, HID=1024, H=16, DH=64).

Sharding: data-parallel over batch, 1 batch per NeuronCore, 8 cores, no
collectives.  The host stages per-core inputs (hT and W pre-cast to bf16,
head-indicator matrix E, bias/count rearranges); all FLOPs run on device:
  q^T = Wq^T @ hT            [1024(out), 2048(s)]   (PE, bf16, fp32 psum)
  k,v = hT^T @ W             [2048(s), 1024(out)]   (PE, bf16, fp32 psum)
  sumsq_q = E^T @ (q^T)^2    (PE) -> rs_q = 1/sqrt(count^2*sumsq + eps)
  kv[h] = k_n[h]^T @ v[h]    [64, 64] per head, accumulated over s-chunks
  ctx    = q^T^T @ kv_blockdiag, rows scaled by rs_q (DVE)
The l2 norm of q and the 1/count on v are folded into the final row scale
(count^2 enters via the Sqrt activation's per-partition scale operand);
the l2 norm of k is applied before the kv matmul.
"""

import numpy as np

import concourse.bacc as bacc
import concourse.mybir as mybir
from concourse import tile
from concourse.bass_utils import run_bass_kernel_spmd

B, S, HID = 8, 2048, 1024
H, DH = 16, 64
P = 128                       # partitions
NG = HID // P                 # 8 column/row groups of 128
NSC = S // P                  # 16 seq chunks of 128
NSJ = S // 512                # 4 seq chunks of 512
KCH = HID // P                # 8 contraction chunks
EPS = 1e-24

F32 = mybir.dt.float32
BF16 = mybir.dt.bfloat16
AF = mybir.ActivationFunctionType


def build(has_kv_bias: bool, debug: bool = False):
    nc = bacc.Bacc("TRN2", target_bir_lowering=False, debug=False, num_devices=8)

    ht_d = nc.dram_tensor("ht", [HID, S], BF16, kind="ExternalInput")
    wq_d = nc.dram_tensor("wq", [HID, HID], BF16, kind="ExternalInput")
    wk_d = nc.dram_tensor("wk", [HID, HID], BF16, kind="ExternalInput")
    wv_d = nc.dram_tensor("wv", [HID, HID], BF16, kind="ExternalInput")
    e_d = nc.dram_tensor("e", [P, NG * H], BF16, kind="ExternalInput")
    i16_d = nc.dram_tensor("i16", [16, 16], F32, kind="ExternalInput")
    vs2_d = nc.dram_tensor("vs2", [P, 1], F32, kind="ExternalInput")
    bqt_d = nc.dram_tensor("bqt", [P, NG], F32, kind="ExternalInput")
    if has_kv_bias:
        bk_d = nc.dram_tensor("bk", [P, HID], F32, kind="ExternalInput")
        bv_d = nc.dram_tensor("bv", [P, HID], F32, kind="ExternalInput")
    out_d = nc.dram_tensor("out", [S, HID], F32, kind="ExternalOutput")
    if debug:
        dbg_qt = nc.dram_tensor("dbg_qt", [HID, S], BF16, kind="ExternalOutput")
        dbg_kn = nc.dram_tensor("dbg_kn", [S, HID], BF16, kind="ExternalOutput")
        dbg_kv = nc.dram_tensor("dbg_kv", [P, H * DH], BF16, kind="ExternalOutput")
        dbg_rs = nc.dram_tensor("dbg_rs", [S, H], F32, kind="ExternalOutput")

    with tile.TileContext(nc) as tc:
        with (
            tc.tile_pool(name="persist", bufs=1) as pp,
            tc.tile_pool(name="work", bufs=3) as wp,
            tc.tile_pool(name="outp", bufs=3) as op,
        ):
            # ---- constants / weights -------------------------------------
            e_sb = pp.tile([P, NG * H], BF16, tag="e")
            nc.sync.dma_start(e_sb[:], e_d[:])
            i16_sb = pp.tile([16, 16], F32, tag="i16")
            nc.sync.dma_start(i16_sb[:], i16_d[:])
            vs2_sb = pp.tile([P, 1], F32, tag="vs2")
            nc.sync.dma_start(vs2_sb[:], vs2_d[:])
            bqt_sb = pp.tile([P, NG], F32, tag="bqt")
            nc.sync.dma_start(bqt_sb[:], bqt_d[:])
            eps_sb = pp.tile([P, 1], F32, tag="eps")
            nc.gpsimd.memset(eps_sb[:], EPS)
            if has_kv_bias:
                bk_sb = pp.tile([P, HID], F32, tag="bk")
                nc.sync.dma_start(bk_sb[:], bk_d[:])
                bv_sb = pp.tile([P, HID], F32, tag="bv")
                nc.sync.dma_start(bv_sb[:], bv_d[:])

            # ---- load hT / W (bf16 in DRAM, host pre-staged) -------------
            # one SWDGE queue, in order: wq + the first hT column block
            # gate the q projection, so they go first
            w_full = {}
            for name in ("q", "k", "v"):
                w_full[name] = pp.tile([P, KCH * HID], BF16, name=f"w{name}",
                                       tag=f"w{name}")
            def _load_w(name, wd, eng):
                for kk in range(KCH):
                    eng.dma_start(
                        w_full[name][:, kk * HID : (kk + 1) * HID],
                        wd[kk * P : (kk + 1) * P, :],
                    )
            hta = pp.tile([P, NG * S], BF16, tag="hta")
            ht = [hta[:, g * S : (g + 1) * S] for g in range(NG)]
            ht3 = hta[:].rearrange("p (kk s) -> p kk s", s=S)
            def _load_ht(g, jh):
                nc.gpsimd.dma_start(
                    ht[g][:, jh * 1024 : (jh + 1) * 1024],
                    ht_d[g * P : (g + 1) * P, jh * 1024 : (jh + 1) * 1024],
                )
            def _load_w1(name, wd, kk):
                nc.gpsimd.dma_start(
                    w_full[name][:, kk * HID : (kk + 1) * HID],
                    wd[kk * P : (kk + 1) * P, :],
                )
            for kk in range(KCH):
                _load_w1("q", wq_d, kk)
                _load_ht(kk, 0)
            for g in range(NG):
                _load_ht(g, 1)
            _load_w("k", wk_d, nc.gpsimd)
            _load_w("v", wv_d, nc.gpsimd)
            w3 = {n: w_full[n][:].rearrange("p (kk o) -> p kk o", o=HID)
                  for n in ("q", "k", "v")}

            # ---- q projection (qT layout) + row sumsq via E-matmul -------
            qt = [pp.tile([P, S], BF16, name=f"qt{g}", tag=f"qt{g}")
                  for g in range(NG)]
            sst = pp.tile([16, S], F32, tag="sst")
            with (
                tc.tile_pool(name="qpsum", bufs=3, space="PSUM") as qps,
                tc.tile_pool(name="sspsum", bufs=2, space="PSUM") as ssps,
            ):
                for j in range(NSJ):
                    ss_ps = ssps.tile([16, 512], F32, tag="ssp")
                    for g in range(NG):
                        q_ps = qps.tile([P, 512], F32, tag="qp")
                        for kk in range(KCH):
                            nc.tensor.matmul(
                                q_ps[:],
                                w3["q"][:, kk, g * P : (g + 1) * P],
                                ht3[:, kk, j * 512 : (j + 1) * 512],
                                start=(kk == 0),
                                stop=(kk == KCH - 1),
                            )
                        # psum -> sbuf bf16 with per-partition bias add
                        nc.scalar.activation(
                            qt[g][:, j * 512 : (j + 1) * 512],
                            q_ps[:],
                            AF.Identity,
                            bias=bqt_sb[:, g : g + 1],
                        )
                        qsq = wp.tile([P, 512], BF16, tag="qsq")
                        nc.vector.tensor_mul(
                            qsq[:],
                            qt[g][:, j * 512 : (j + 1) * 512],
                            qt[g][:, j * 512 : (j + 1) * 512],
                        )
                        nc.tensor.matmul(
                            ss_ps[:],
                            e_sb[:, g * H : (g + 1) * H],
                            qsq[:],
                            start=(g == 0),
                            stop=(g == NG - 1),
                        )
                    nc.scalar.copy(sst[:, j * 512 : (j + 1) * 512], ss_ps[:])

            # rs_q[:, sc*H:(sc+1)*H] = 1 / sqrt(count^2 * sumsq + eps)
            rs_all = pp.tile([P, NSC * H], F32, tag="rsall")
            rs_q = [rs_all[:, sc * H : (sc + 1) * H] for sc in range(NSC)]
            with tc.tile_pool(name="rspsum", bufs=2, space="PSUM") as rsps:
                for sc in range(NSC):
                    rs_ps = rsps.tile([P, 16], F32, tag="rsp")
                    nc.tensor.transpose(
                        rs_ps[:], sst[:, sc * P : (sc + 1) * P], i16_sb[:]
                    )
                    sq = wp.tile([P, H], F32, tag="sqq")
                    nc.scalar.activation(
                        sq[:],
                        rs_ps[:],
                        AF.Sqrt,
                        bias=eps_sb[:, 0:1],
                        scale=vs2_sb[:, 0:1],
                    )
                    nc.vector.reciprocal(rs_q[sc][:], sq[:])
                    if debug:
                        nc.sync.dma_start(
                            dbg_rs[sc * P : (sc + 1) * P, :], rs_q[sc][:]
                        )

            # ---- k/v projections + kv accumulation -----------------------
            kv_sb = pp.tile([P, H * DH], BF16, tag="kvsb")
            nc.vector.memset(kv_sb[:], 0.0)
            with (
                tc.tile_pool(name="kvproj", bufs=3, space="PSUM") as kvp,
                tc.tile_pool(name="kvacc", bufs=1, space="PSUM") as kva,
            ):
                kv_pse = kva.tile([DH, H * 32], F32, name="kvacc0", tag="kvacc0")
                kv_pso = kva.tile([DH, H * 32], F32, name="kvacc1", tag="kvacc1")
                for sc in range(NSC):
                    k_ps = [kvp.tile([P, 512], F32, name=f"kp{sc}_{i}", tag="kp")
                            for i in range(2)]
                    v_ps = [kvp.tile([P, 512], F32, name=f"vp{sc}_{i}", tag="vp")
                            for i in range(2)]
                    for nj in range(2):
                        for kk in range(KCH):
                            nc.tensor.matmul(
                                k_ps[nj][:],
                                ht3[:, kk, sc * P : (sc + 1) * P],
                                w3["k"][:, kk, nj * 512 : (nj + 1) * 512],
                                start=(kk == 0),
                                stop=(kk == KCH - 1),
                            )
                        for kk in range(KCH):
                            nc.tensor.matmul(
                                v_ps[nj][:],
                                ht3[:, kk, sc * P : (sc + 1) * P],
                                w3["v"][:, kk, nj * 512 : (nj + 1) * 512],
                                start=(kk == 0),
                                stop=(kk == KCH - 1),
                            )
                        if has_kv_bias:
                            nc.vector.tensor_add(
                                k_ps[nj][:], k_ps[nj][:],
                                bk_sb[:, nj * 512 : (nj + 1) * 512],
                            )
                            nc.vector.tensor_add(
                                v_ps[nj][:], v_ps[nj][:],
                                bv_sb[:, nj * 512 : (nj + 1) * 512],
                            )

                    # row sumsq of k per head -> rs_k -> k_n; v -> bf16
                    ksq = wp.tile([P, HID], BF16, tag="ksq")
                    ssk = wp.tile([P, H], F32, tag="ssk")
                    sqk = wp.tile([P, H], F32, tag="sqk")
                    rsk = wp.tile([P, H], F32, tag="rsk")
                    k_n = wp.tile([P, HID], BF16, tag="kn")
                    v_sb = wp.tile([P, HID], BF16, tag="vsb")
                    for nj in range(2):
                        sl = slice(nj * 512, (nj + 1) * 512)
                        nc.scalar.activation(ksq[:, sl], k_ps[nj][:], AF.Square)
                        nc.scalar.copy(v_sb[:, sl], v_ps[nj][:])
                    nc.vector.tensor_reduce(
                        ssk[:],
                        ksq[:].rearrange("p (h d) -> p h d", d=DH),
                        axis=mybir.AxisListType.X,
                        op=mybir.AluOpType.add,
                    )
                    nc.scalar.activation(
                        sqk[:], ssk[:], AF.Sqrt, bias=eps_sb[:, 0:1]
                    )
                    nc.vector.reciprocal(rsk[:], sqk[:])
                    for nj in range(2):
                        nc.vector.tensor_mul(
                            k_n[:].rearrange("p (h d) -> p h d", d=DH)[
                                :, nj * 8 : (nj + 1) * 8, :
                            ],
                            k_ps[nj][:].rearrange("p (h d) -> p h d", d=DH),
                            rsk[:, nj * 8 : (nj + 1) * 8, None].broadcast_to(
                                [P, 8, DH]
                            ),
                        )
                    if debug:
                        nc.sync.dma_start(dbg_kn[sc * P : (sc + 1) * P, :], k_n[:])

                    # kv accumulation: even heads in kv_pse, odd in kv_pso;
                    # one accumulation group per bank: heads 0/1 open it,
                    # heads 14/15 close it
                    for hh in range(H):
                        kv_ps = kv_pse if hh % 2 == 0 else kv_pso
                        co = (hh // 2) * DH
                        nc.tensor.matmul(
                            kv_ps[:, co : co + DH],
                            k_n[:, hh * DH : (hh + 1) * DH],
                            v_sb[:, hh * DH : (hh + 1) * DH],
                            start=(sc == 0 and hh < 2),
                            stop=(sc == NSC - 1 and hh >= H - 2),
                        )
                kvv = kv_sb[:].rearrange("p (pp two d) -> p pp two d",
                                          two=2, d=DH)
                for hp2 in range(2):
                    pps = slice(hp2 * 4, (hp2 + 1) * 4)
                    nc.scalar.copy(
                        kvv[0:DH, pps, 0, :],
                        kv_pse[:].rearrange("p (pp d) -> p pp d", d=DH)[
                            :, pps, :
                        ],
                    )
                    nc.scalar.copy(
                        kvv[DH:P, pps, 1, :],
                        kv_pso[:].rearrange("p (pp d) -> p pp d", d=DH)[
                            :, pps, :
                        ],
                    )
                if debug:
                    nc.sync.dma_start(dbg_kv[:], kv_sb[:])

            # ---- ctx = qT^T @ kv (block-diagonal), scaled by rs_q --------
            # kv_sb is block-diagonal per head pair, so one K=128 matmul
            # computes both heads of a pair; heads land in natural column
            # order (pair-major, even|odd)
            with tc.tile_pool(name="ctxpsum", bufs=2, space="PSUM") as cps:
                for s2 in range(NSC // 2):
                    out_t = op.tile([P, 2 * HID], F32, tag="outt")
                    c_ps = cps.tile([P, 2 * HID], F32, name=f"cp{s2}", tag="cp")
                    for i in range(2):
                        sc = s2 * 2 + i
                        for pair in range(8):
                            nc.tensor.matmul(
                                c_ps[:, i * HID + pair * P :
                                     i * HID + (pair + 1) * P],
                                qt[pair][:, sc * P : (sc + 1) * P],
                                kv_sb[:, pair * P : (pair + 1) * P],
                                start=True,
                                stop=True,
                            )
                    nc.vector.tensor_mul(
                        out_t[:].rearrange("p (h d) -> p h d", d=DH),
                        c_ps[:].rearrange("p (h d) -> p h d", d=DH),
                        rs_all[:, s2 * 2 * H : (s2 * 2 + 2) * H][
                            :, :, None
                        ].broadcast_to([P, 2 * H, DH]),
                    )
                    for i in range(2):
                        sc = s2 * 2 + i
                        oeng = nc.sync if sc % 2 == 0 else nc.scalar
                        oeng.dma_start(
                            out_d[sc * P : (sc + 1) * P, :],
                            out_t[:, i * HID : (i + 1) * HID],
                        )
                    if debug:
                        for g in range(NG):
                            nc.sync.dma_start(
                                dbg_qt[g * P : (g + 1) * P, sc * P : (sc + 1) * P],
                                qt[g][:, sc * P : (sc + 1) * P],
                            )

    nc.compile()
    return nc


_CACHE = {}


def _get_nc(has_kv_bias: bool, debug: bool = False):
    key = (has_kv_bias, debug)
    if key not in _CACHE:
        _CACHE[key] = build(has_kv_bias, debug)
    return _CACHE[key]


def _prep_inputs(hidden_states, attention_mask, Wq, bq, Wk, bk, Wv, bv):
    """Host-side shard prep. Returns (in_maps, has_kv_bias)."""
    hs = np.asarray(hidden_states, dtype=np.float32)
    am = np.asarray(attention_mask)
    m = (am == 0).astype(np.float32).reshape(B, S)      # [B, S] valid mask
    counts = m.sum(axis=1)                               # [B]
    if not np.all(m == 1.0):
        hs = hs * m[:, :, None]                          # exact when biases==0

    wq = np.asarray(Wq, dtype=np.float32)
    wk = np.asarray(Wk, dtype=np.float32)
    wv = np.asarray(Wv, dtype=np.float32)
    bq_ = np.asarray(bq, dtype=np.float32)
    bk_ = np.asarray(bk, dtype=np.float32)
    bv_ = np.asarray(bv, dtype=np.float32)
    has_kv_bias = bool(np.any(bk_ != 0) or np.any(bv_ != 0))

    import ml_dtypes

    wq16 = wq.astype(ml_dtypes.bfloat16)
    wk16 = wk.astype(ml_dtypes.bfloat16)
    wv16 = wv.astype(ml_dtypes.bfloat16)

    # E[p, g*H + h] = 1 if hid index g*128+p belongs to head h
    o = np.arange(HID)
    e_full = (o[:, None] // DH == np.arange(H)[None, :]).astype(np.float32)
    e_np = np.ascontiguousarray(
        e_full.reshape(NG, P, H).transpose(1, 0, 2).reshape(P, NG * H)
    ).astype(ml_dtypes.bfloat16)
    i16 = np.eye(16, dtype=np.float32)
    bqt = np.ascontiguousarray(bq_.reshape(NG, P).T)     # [128, 8]

    in_maps = []
    for b in range(B):
        im = {
            "ht": np.ascontiguousarray(hs[b].T).astype(ml_dtypes.bfloat16),
            "wq": wq16, "wk": wk16, "wv": wv16,
            "e": e_np, "i16": i16,
            "vs2": np.full((P, 1), np.float32(counts[b]) ** 2, np.float32),
            "bqt": bqt,
        }
        if has_kv_bias:
            im["bk"] = np.broadcast_to(bk_, (P, HID)).copy()
            im["bv"] = np.broadcast_to(bv_, (P, HID)).copy()
        in_maps.append(im)
    return in_maps, has_kv_bias


def run(inputs: dict, trace: bool = False, debug: bool = False):
    in_maps, has_kv_bias = _prep_inputs(**inputs)
    nc = _get_nc(has_kv_bias, debug)
    res = run_bass_kernel_spmd(nc, in_maps, list(range(B)), trace=trace)
    out = np.stack([res.results[i]["out"] for i in range(B)]).astype(np.float32)
    return out, res


def kernel(**inputs) -> np.ndarray:
    out, _ = run(inputs)
    return out
